# revision 41
# baseline (speedup 1.0000x reference)
"""BEiT-style windowed attention with relative position bias, on 8 trn2 cores.

Sharding: data-parallel over batch (32 batches -> 4 per core). Weights and the
host-gathered exp(bias) table are replicated.

Device pipeline per core (matmuls bf16 in / f32 accumulate):
  Phase 1: qkT = Wqk @ x^T  -> [1536, 2308] (q pre-scaled by 1/8),
           v   = x @ Wv^T   -> per (batch, row-tile) tiles with an appended
                               ones column per head (softmax denominators).
  Phase 2 per (head, batch), software-pipelined one iteration deep:
    scores^T [j, i] via 10 single-shot K=64 matmuls into TWO psum tiles
    (T-A: j-tiles 0-1; T-B: j-tiles 2-4 + all i>=512 columns) -> exp on
    ScalarE per tile (the tile split lets the next iteration's QK chase the
    exps tile-by-tile; whole-tile dependency tracking would otherwise
    serialize) -> E = E0 * exp(bias) elementwise on DVE+Pool (bias folded
    multiplicatively: exp(s+b) = exp(s)*exp(b)) -> PV with E stationary ->
    O[i, d] + denominator column -> per-partition reciprocal+normalize on
    DVE -> XBAR DMA transposes -> 2 batched Pool copies -> O^T tiles.
    The ten q/k projection m-tiles not needed immediately are computed here
    too: 256-wide chunks slotted into PE slack between the QK groups on a
    deadline-aware schedule, through a dedicated psum bank with paired evacs.
  Phase 3: out^T = Wp^T.T @ O^T + b -> DMA out, host transposes back.

  ScalarE's exp is the phase-2 rate limiter (no 16-bit speedup on Act), so
  everything else is arranged to hide under it; Pool gets only SBUF-to-SBUF
  work (hardware forbids GPSIMD PSUM access) and PSUM evacuations go to
  DVE/Act with partition bases kept at 0/64.
"""

import numpy as np
import ml_dtypes

import concourse.bass as bass
import concourse.tile as tile
from concourse import bacc, mybir
from concourse.bass_utils import run_bass_kernel_spmd

BF16 = mybir.dt.bfloat16
F32 = mybir.dt.float32
AF = mybir.ActivationFunctionType

NCORES = 8
B = 32
BPC = B // NCORES          # batches per core
N = 577                    # sequence length
C = 768
H = 12
HD = 64
R = BPC * N                # rows per core (2308)
CT = C // 128              # 6 contraction tiles
MT = 12                    # qk output row-tiles (1536/128)
JTS = [128, 128, 128, 128, 65]   # j tiles of N
ACOLS = 1024               # T-A: j-tiles 0,1 x i 0:512
BCOLS = 1861               # T-B: j-tiles 2,3,4 x i 0:512 (1536) + c1 5*65
ECOLS = ACOLS + BCOLS      # 2885 packed score/E columns
RCHUNKS = [(0, 512), (512, 512), (1024, 512), (1536, 512), (2048, 260)]
MULB = 768                 # DVE does E-B mul cols [0:MULB], Pool the rest

_PROGRAM = None


def build_program():
    nc = bacc.Bacc(trn_type="TRN2", name="beit_attn")

    xt_d = nc.dram_tensor("xt", [128, CT * R], BF16, kind="ExternalInput")
    # q/k weights m-major ([128, 12 m-tiles x (6 kk x 128)]), v weights kk-major
    wqk_d = nc.dram_tensor("wqk", [128, MT * C], BF16, kind="ExternalInput")
    wv_d = nc.dram_tensor("wv", [128, CT * C], BF16, kind="ExternalInput")
    wp_d = nc.dram_tensor("wp", [128, CT * C], BF16, kind="ExternalInput")
    eb_d = nc.dram_tensor("expb", [H, 128, ECOLS], BF16, kind="ExternalInput")
    id_d = nc.dram_tensor("ident", [128, 128], BF16, kind="ExternalInput")
    pb_d = nc.dram_tensor("pb", [128, CT], F32, kind="ExternalInput")
    out_d = nc.dram_tensor("ftout", [128, CT * R], F32, kind="ExternalOutput")

    with tile.TileContext(nc) as tc:
        with (
            tc.tile_pool(name="static", bufs=1) as sp,
            tc.tile_pool(name="qk", bufs=1) as qk_pool,
            tc.tile_pool(name="v1", bufs=1) as v1_pool,
        ):
            ident = sp.tile([128, 128], BF16, tag="ident")
            pb = sp.tile([128, CT], F32, tag="pb")
            wp_sb = sp.tile([128, CT * C], BF16, tag="wp")
            ot_sb = sp.tile([128, CT * R], BF16, tag="ot")
            # First head's exp-bias staged from the long-lived pool so phase 2
            # doesn't wait on the phase-1 pool-release barrier. DMAs for these
            # are emitted below, sequenced so the phase-1 inputs land first
            # (the DMA fabric is a single shared-bandwidth resource).
            eb0_sb = sp.tile([128, ECOLS], BF16, tag="eb0")

            qk_t = [qk_pool.tile([128, R], BF16, tag=f"qk{m}", name=f"qk{m}") for m in range(MT)]
            v1_t = [
                [v1_pool.tile([128, 780], BF16, tag=f"v1_{b}_{t}", name=f"v1_{b}_{t}") for t in range(5)]
                for b in range(BPC)
            ]

            # ---------------- Phase 1: QKV projections -------------------
            # p1b holds what the interleaved qkT tail (emitted inside phase 2)
            # still needs: x chunks + the last four q/k weight tiles.
            LATE_M = (1, 7, 2, 8, 3, 9, 4, 10, 5, 11)
            p1b_cm = tc.tile_pool(name="ph1b", bufs=1)
            p1b = p1b_cm.__enter__()
            xt_sb = [
                [
                    p1b.tile([128, 1089], BF16, tag=f"xta{kk}", name=f"xta{kk}"),
                    p1b.tile([128, R - 1024], BF16, tag=f"xtb{kk}", name=f"xtb{kk}"),
                ]
                for kk in range(CT)
            ]

            # the halves overlap on cols 1024:1089 so no engine read straddles
            def xt_ap(kk, c0, cn):
                if c0 + cn <= 1089:
                    return xt_sb[kk][0][:, c0 : c0 + cn]
                return xt_sb[kk][1][:, c0 - 1024 : c0 - 1024 + cn]
            wqk_sb = {}
            for m in LATE_M:
                wqk_sb[m] = p1b.tile([128, C], BF16, tag=f"wqk{m}", name=f"wqk{m}")
            with (
                tc.tile_pool(name="ph1", bufs=1) as p1,
                tc.tile_pool(name="psum1", bufs=4, space="PSUM") as psum1,
            ):
                for m in range(MT):
                    if m not in wqk_sb:
                        wqk_sb[m] = p1.tile([128, C], BF16, tag=f"wqk{m}", name=f"wqk{m}")
                wv_sb = [
                    p1.tile([128, C], BF16, tag=f"wv{kk}", name=f"wv{kk}")
                    for kk in range(CT)
                ]

                def dma_wqk(m):
                    nc.sync.dma_start(wqk_sb[m][:], wqk_d[:, C * m : C * (m + 1)])

                # DMA order = arrival order; transfers issued from SP and
                # Act run on separate queues concurrently, so the critical
                # inputs are split across both engines.
                dma_wqk(0)
                nc.scalar.dma_start(wqk_sb[6][:], wqk_d[:, C * 6 : C * 7])
                for kk in range(CT):
                    nc.sync.dma_start(xt_sb[kk][0][:], xt_d[:, R * kk : R * kk + 1089])
                    nc.scalar.dma_start(
                        xt_sb[kk][1][:], xt_d[:, R * kk + 1024 : R * (kk + 1)]
                    )
                for kk in range(CT):
                    eng = nc.sync if kk % 2 == 0 else nc.scalar
                    eng.dma_start(wv_sb[kk][:], wv_d[:, C * kk : C * (kk + 1)])
                nc.scalar.dma_start(eb0_sb[:], eb_d[0])
                nc.sync.dma_start(ident[:], id_d[:])
                nc.sync.dma_start(pb[:], pb_d[:])
                for i, m in enumerate(LATE_M):
                    eng = nc.sync if i % 2 == 0 else nc.scalar
                    eng.dma_start(wqk_sb[m][:], wqk_d[:, C * m : C * (m + 1)])
                nc.sync.dma_start(wp_sb[:], wp_d[:])

                evac_flip = [0]

                def emit_qkT(m):
                    for c0, cn in RCHUNKS:
                        ps = psum1.tile([128, 512], F32, tag="ps1")
                        for kk in range(CT):
                            nc.tensor.matmul(
                                ps[:, :cn],
                                wqk_sb[m][:, 128 * kk : 128 * (kk + 1)],
                                xt_ap(kk, c0, cn),
                                start=(kk == 0),
                                stop=(kk == CT - 1),
                            )
                        evac_flip[0] ^= 1
                        if evac_flip[0]:
                            nc.vector.tensor_copy(qk_t[m][:, c0 : c0 + cn], ps[:, :cn])
                        else:
                            nc.scalar.copy(qk_t[m][:, c0 : c0 + cn], ps[:, :cn])

                def emit_v(b):
                    for t in range(5):
                        kj = JTS[t]
                        for vc0, vcn, h0 in ((0, 512, 0), (512, 256, 8)):
                            ps = psum1.tile([128, 512], F32, tag="ps1")
                            for kk in range(CT):
                                nc.tensor.matmul(
                                    ps[:kj, :vcn],
                                    xt_ap(kk, N * b + 128 * t, kj),
                                    wv_sb[kk][:, vc0 : vc0 + vcn],
                                    start=(kk == 0),
                                    stop=(kk == CT - 1),
                                )
                            dest = v1_t[b][t][:kj, 65 * h0 : 65 * (h0 + vcn // 64)]
                            dest = dest.rearrange("p (h d) -> p h d", d=65)[:, :, 0:64]
                            src = ps[:kj, :vcn].rearrange("p (h d) -> p h d", d=64)
                            nc.vector.tensor_copy(dest, src)
                        ones_ap = v1_t[b][t][:, :].rearrange("p (h d) -> p h d", d=65)[
                            :, :, 64:65
                        ]
                        nc.gpsimd.memset(ones_ap, 1.0)

                # q/k for head pair 0, then v; the late m-tiles (LATE_M)
                # run inside phase 2's PE slack.
                for m in (0, 6):
                    emit_qkT(m)
                for b in range(BPC):
                    emit_v(b)

            # ---------------- Phase 2: attention -------------------------
            with (
                tc.tile_pool(name="ph2", bufs=2) as p2,
                tc.tile_pool(name="ph2sb", bufs=1) as score_pool_sb,
                tc.tile_pool(name="score", bufs=1, space="PSUM") as score_pool,
                tc.tile_pool(name="pvp", bufs=1, space="PSUM") as pv_pool,
                tc.tile_pool(name="ps1b", bufs=1, space="PSUM") as ps1b_pool,
            ):
                ps1b = ps1b_pool.tile([128, 512], F32, tag="ps1b")
                # half-width chunks keep each insertion small; deadline-aware
                # greedy plan: all chunks of head-pair hp must be evacuated
                # before iteration 8*hp reads them.
                RCH2 = [(c, 256) for c in range(0, 2048, 256)] + [(2048, 130), (2178, 130)]
                late_chunks = [
                    (8 * (m % 6) - (2 if m % 6 <= 2 else 3), m, c0, cn, i % 2)
                    for m in LATE_M
                    for i, (c0, cn) in enumerate(RCH2)
                ]
                late_chunks.sort(key=lambda t: t[0])
                plan = [0] * 48
                for dl, _m, _c, _n, _p in late_chunks:
                    slot = min(dl, 47)
                    while slot >= 0 and plan[slot] >= 3:
                        slot -= 1
                    assert slot >= 0, "interleave schedule infeasible"
                    plan[slot] += 1
                # smooth: pull work earlier (never later, so deadlines hold)
                for idx in range(48):
                    while plan[idx] > 2:
                        cands = [j for j in range(idx) if plan[j] < 2]
                        if not cands:
                            break
                        plan[cands[0]] += 1
                        plan[idx] -= 1
                late_flip = [0]

                pend = [None]

                def emit_late_chunk():
                    if not late_chunks:
                        return
                    _dl, m, c0, cn, par = late_chunks.pop(0)
                    pcol = 0 if par == 0 else pend[0][2]
                    for kk in range(CT):
                        nc.tensor.matmul(
                            ps1b[:, pcol : pcol + cn],
                            wqk_sb[m][:, 128 * kk : 128 * (kk + 1)],
                            xt_ap(kk, c0, cn),
                            start=(kk == 0),
                            stop=(kk == CT - 1),
                        )
                    if par == 0:
                        pend[0] = (m, c0, cn)
                    else:
                        pm, pc0, pcn = pend[0]
                        nc.vector.tensor_copy(
                            qk_t[m][:, pc0 : pc0 + pcn + cn],
                            ps1b[:, 0 : pcn + cn],
                        )
                TA = score_pool.tile([128, ACOLS], F32, tag="TA")
                TB = score_pool.tile([128, 2048], F32, tag="TB")
                pv = pv_pool.tile([128, 512], F32, tag="pv")
                # One-time init: j-tile-4 regions rows 65:128 are never written
                # by the M=65 matmuls; zero them so exp stays finite (the expb
                # j-padding rows are 0 so E becomes 0 there). pv rows 65:128 of
                # the s=4 group never get written; 1.0 keeps the strided
                # reciprocal finite. stg pad columns feed the XBAR transpose,
                # so they must be finite too; both rotating buffers get one
                # memset.
                nc.vector.memset(TB[64:128, 1024:1536], 0.0)
                nc.vector.memset(TB[64:128, 1796:1861], 0.0)
                nc.vector.memset(pv[64:128, 260:325], 1.0)
                stg_t = [score_pool_sb.tile([128, 640], BF16, tag=f"stg{i}", name=f"stg{i}") for i in range(2)]
                xscr_t = [score_pool_sb.tile([128, 640], BF16, tag=f"xscr{i}", name=f"xscr{i}") for i in range(2)]
                for i in range(2):
                    nc.gpsimd.memset(stg_t[i][:, :], 0.0)

                prev = [None]

                def emit_tail(st):
                    h, b, par, E0A, E0B, eb_t = st
                    qp = 64 * (h % 2)
                    # E = E0 * exp(bias): A-half on DVE, B-half split DVE/Pool
                    EA = score_pool_sb.tile([128, ACOLS], BF16, tag="EA")
                    EB = score_pool_sb.tile([128, BCOLS], BF16, tag="EB")
                    nc.vector.tensor_mul(EA[:, :], E0A[:, :], eb_t[:, 0:ACOLS])
                    # B-half split jt-aligned so PV's j-tile-2 matmul unblocks
                    # as soon as its own slice is multiplied.
                    nc.gpsimd.tensor_mul(
                        EB[:, 0:512], E0B[:, 0:512], eb_t[:, ACOLS : ACOLS + 512]
                    )
                    nc.gpsimd.tensor_mul(
                        EB[:, 512:1024], E0B[:, 512:1024], eb_t[:, ACOLS + 512 : ACOLS + 1024]
                    )
                    nc.gpsimd.tensor_mul(
                        EB[:, 1024:BCOLS], E0B[:, 1024:BCOLS], eb_t[:, ACOLS + 1024 : ECOLS]
                    )
                    # PV: E stationary, [v|1] moving -> O[i, 65] per i-tile
                    for s in range(5):
                        ki = JTS[s]
                        for jt in range(5):
                            kj = JTS[jt]
                            if s < 4:
                                lhs = (
                                    EA[:kj, 512 * jt + 128 * s : 512 * jt + 128 * s + ki]
                                    if jt < 2
                                    else EB[
                                        :kj,
                                        512 * (jt - 2) + 128 * s : 512 * (jt - 2) + 128 * s + ki,
                                    ]
                                )
                            else:
                                lhs = EB[:kj, 1536 + 65 * jt : 1536 + 65 * jt + ki]
                            nc.tensor.matmul(
                                pv[:ki, 65 * s : 65 * s + 65],
                                lhs,
                                v1_t[b][jt][:kj, 65 * h : 65 * h + 65],
                                start=(jt == 0),
                                stop=(jt == 4),
                            )
                    # per-partition denominators -> reciprocal -> normalize
                    rcp = p2.tile([128, 8], F32, tag="rcp")
                    den_ap = pv[:, 0:325].rearrange("p (s c) -> p s c", c=65)[:, :, 64:65]
                    nc.vector.reciprocal_approx_fast(rcp[:, 0:5], den_ap)
                    # stg groups are 128 wide (64 valid + 64 memset pad) so the
                    # XBAR DMA transpose gets its 128-col alignment. Persistent
                    # parity-rotated tiles keep the pad memset valid.
                    stg = stg_t[par]
                    for s in range(5):
                        ki = JTS[s]
                        nc.vector.tensor_scalar_mul(
                            stg[:ki, 128 * s : 128 * s + 64],
                            pv[:ki, 65 * s : 65 * s + 64],
                            rcp[:ki, s : s + 1],
                        )
                    # O[i, d] -> O^T[d, i] via DMA crossbar transpose into an
                    # SBUF scratch (garbage in rows 64:128 from the pad cols),
                    # then 2 batched Pool copies of the valid rows into ot_sb.
                    obase = R * (h // 2) + N * b
                    xscr = xscr_t[par]
                    for s in range(5):
                        nc.sync.dma_start_transpose(
                            xscr[:, 128 * s : 128 * (s + 1)],
                            stg[:, 128 * s : 128 * (s + 1)],
                        )
                    xv = xscr[:, :].rearrange("p (s c) -> p s c", c=128)
                    nc.gpsimd.tensor_copy(
                        ot_sb[qp : qp + 64, obase : obase + 512],
                        xv[0:64, 0:4, :],
                    )
                    nc.gpsimd.tensor_copy(
                        ot_sb[qp : qp + 64, obase + 512 : obase + 577],
                        xscr[0:64, 512:577],
                    )

                for h in range(H):
                    if h == 0:
                        eb_t = eb0_sb
                    else:
                        eb_t = p2.tile([128, ECOLS], BF16, tag="eb")
                        nc.sync.dma_start(eb_t[:], eb_d[h])
                    qp = 64 * (h % 2)
                    qm, km = h // 2, 6 + h // 2
                    for b in range(BPC):
                        q_c0 = qk_t[qm][qp : qp + 64, N * b : N * b + 512]
                        q_c1 = qk_t[qm][qp : qp + 64, N * b + 512 : N * b + 577]
                        kTs = [
                            qk_t[km][qp : qp + 64, N * b + 128 * jt : N * b + 128 * jt + JTS[jt]]
                            for jt in range(5)
                        ]
                        # T-A: j-tiles 0,1 c0
                        for jt in range(2):
                            nc.tensor.matmul(
                                TA[: JTS[jt], 512 * jt : 512 * jt + 512], kTs[jt], q_c0,
                                start=True, stop=True,
                            )
                        idx = 4 * h + b
                        nslots = plan[idx]
                        if nslots >= 1:
                            emit_late_chunk()
                        # T-B: j-tiles 2,3,4 c0 + all c1
                        for jt in range(2, 5):
                            nc.tensor.matmul(
                                TB[: JTS[jt], 512 * (jt - 2) : 512 * (jt - 2) + 512],
                                kTs[jt], q_c0,
                                start=True, stop=True,
                            )
                        for jt in range(5):
                            nc.tensor.matmul(
                                TB[: JTS[jt], 1536 + 65 * jt : 1536 + 65 * jt + 65],
                                kTs[jt], q_c1,
                                start=True, stop=True,
                            )
                        for _ in range(nslots - 1):
                            emit_late_chunk()
                        E0A = p2.tile([128, ACOLS], BF16, tag="E0A")
                        E0B = p2.tile([128, BCOLS], BF16, tag="E0B")
                        nc.scalar.activation(E0A[:, :], TA[:, :], AF.Exp)
                        nc.scalar.activation(E0B[:, :], TB[:, 0:BCOLS], AF.Exp)
                        if prev[0] is not None:
                            emit_tail(prev[0])
                        prev[0] = (h, b, (4 * h + b) % 2, E0A, E0B, eb_t)
                emit_tail(prev[0])

            p1b_cm.__exit__(None, None, None)

            # ---------------- Phase 3: output projection -----------------
            with (
                tc.tile_pool(name="ph3", bufs=6) as p3,
                tc.tile_pool(name="psum3", bufs=8, space="PSUM") as psum3,
            ):
                for m in range(CT):
                    for c0, cn in RCHUNKS:
                        ps = psum3.tile([128, 512], F32, tag="ps3")
                        for kk in range(CT):
                            nc.tensor.matmul(
                                ps[:, :cn],
                                wp_sb[:, C * kk + 128 * m : C * kk + 128 * (m + 1)],
                                ot_sb[:, R * kk + c0 : R * kk + c0 + cn],
                                start=(kk == 0),
                                stop=(kk == CT - 1),
                            )
                        ft = p3.tile([128, 512], F32, tag="ft")
                        nc.scalar.add(ft[:, :cn], ps[:, :cn], pb[:, m : m + 1])
                        deng = nc.sync if (m + c0 // 512) % 2 == 0 else nc.scalar
                        deng.dma_start(
                            out_d[:, R * m + c0 : R * m + c0 + cn], ft[:, :cn]
                        )

    nc.compile()
    return nc


def get_program():
    global _PROGRAM
    if _PROGRAM is None:
        _PROGRAM = build_program()
    return _PROGRAM


def _pack_ctiles(a):
    """[768, X] -> SBUF image [128, 6*X] (c-tile kk at cols kk*X..(kk+1)*X)."""
    rows, cols = a.shape
    assert rows == 768
    return np.ascontiguousarray(
        a.reshape(CT, 128, cols).transpose(1, 0, 2).reshape(128, CT * cols)
    )


def make_host_inputs(x, qkv_w, table, rel_index, proj_w, proj_b):
    bf = ml_dtypes.bfloat16
    x = np.asarray(x, np.float32)
    qkv_w = np.asarray(qkv_w, np.float32)
    table = np.asarray(table, np.float32)
    rel_index = np.asarray(rel_index)
    proj_w = np.asarray(proj_w, np.float32)
    proj_b = np.asarray(proj_b, np.float32)

    qkv_ws = qkv_w.copy()
    qkv_ws[:768] *= 0.125                                    # fold q scale (exact in bf16)
    wqkv = _pack_ctiles(qkv_ws.T)                            # [128, 6*2304]
    # q/k weights m-major: tile m holds cols [6 kk x 128] for output rows
    # 128m..128(m+1); v weights stay kk-major.
    wqkv3 = wqkv.reshape(128, CT, 18, 128)
    wqk = np.ascontiguousarray(
        wqkv3[:, :, :MT].transpose(0, 2, 1, 3).reshape(128, MT * C)
    ).astype(bf)
    wv = np.ascontiguousarray(wqkv3[:, :, MT:].reshape(128, CT * C)).astype(bf)
    wp = _pack_ctiles(proj_w.T).astype(bf)                   # [128, 6*768]
    pb = np.ascontiguousarray(proj_b.reshape(CT, 128).T)     # [128, 6]
    ident = np.eye(128, dtype=bf)

    # exp(bias), transposed orientation: ebT[h, j, i] = exp(table[rel_index[i, j], h])
    g = table[rel_index.reshape(-1)].reshape(N, N, H)        # [i, j, h]
    bt = np.exp(g.transpose(2, 1, 0))                        # [h, j, i]
    btp = np.zeros((H, 640, N), np.float32)                  # j-padding rows stay 0
    btp[:, :N] = bt
    btp = btp.reshape(H, 5, 128, N)
    c0 = btp[:, :, :, 0:512].transpose(0, 2, 1, 3).reshape(H, 128, 2560)
    c1 = btp[:, :, :, 512:577].transpose(0, 2, 1, 3).reshape(H, 128, 325)
    # device column order: [jt0 c0, jt1 c0 | jt2-4 c0, c1 x5] = A(1024) + B(1861)
    expb = np.ascontiguousarray(
        np.concatenate([c0[:, :, 0:1024], c0[:, :, 1024:2560], c1], axis=2)
    ).astype(bf)

    in_maps = []
    for c in range(NCORES):
        xT = x[BPC * c : BPC * (c + 1)].reshape(R, C).T      # [768, 2308]
        in_maps.append(
            {
                "xt": _pack_ctiles(xT).astype(bf),
                "wqk": wqk,
                "wv": wv,
                "wp": wp,
                "expb": expb,
                "ident": ident,
                "pb": pb,
            }
        )
    return in_maps


def unpack_output(ft):
    """[128, 6*2308] f32 -> [BPC, 577, 768]."""
    f = ft.reshape(128, CT, R).transpose(1, 0, 2).reshape(C, R)  # [768, 2308]
    return np.ascontiguousarray(f.T).reshape(BPC, N, C)


def kernel(x, qkv_w, table, rel_index, proj_w, proj_b):
    nc = get_program()
    in_maps = make_host_inputs(x, qkv_w, table, rel_index, proj_w, proj_b)
    res = run_bass_kernel_spmd(nc, in_maps, core_ids=list(range(NCORES)))
    out = np.empty((B, N, C), np.float32)
    for c in range(NCORES):
        out[BPC * c : BPC * (c + 1)] = unpack_output(res.results[c]["ftout"])
    return out


# revision 43
# speedup vs baseline: 1.5373x; 1.5373x over previous
"""BEiT-style windowed attention with relative position bias, on 8 trn2 cores.

Sharding: data-parallel over batch (32 batches -> 4 per core). Weights and the
host-gathered exp(bias) table are replicated.

Device pipeline per core (matmuls bf16 in / f32 accumulate):
  Phase 1: qkT = Wqk @ x^T  -> [1536, 2308] (q pre-scaled by 1/8),
           v   = x @ Wv^T   -> per (batch, row-tile) tiles with an appended
                               ones column per head (softmax denominators).
  Phase 2 per (head, batch), software-pipelined one iteration deep:
    scores^T [j, i] via 10 single-shot K=64 matmuls into TWO psum tiles
    (T-A: j-tiles 0-1; T-B: j-tiles 2-4 + all i>=512 columns) -> exp on
    ScalarE per tile (the tile split lets the next iteration's QK chase the
    exps tile-by-tile; whole-tile dependency tracking would otherwise
    serialize) -> E = E0 * exp(bias) elementwise on DVE+Pool (bias folded
    multiplicatively: exp(s+b) = exp(s)*exp(b)) -> PV with E stationary ->
    O[i, d] + denominator column -> per-partition reciprocal+normalize on
    DVE -> XBAR DMA transposes -> 2 batched Pool copies -> O^T tiles.
    The ten q/k projection m-tiles not needed immediately are computed here
    too: 256-wide chunks slotted into PE slack between the QK groups on a
    deadline-aware schedule, through a dedicated psum bank with paired evacs.
  Phase 3: out^T = Wp^T.T @ O^T + b -> DMA out, host transposes back.

  ScalarE's exp is the phase-2 rate limiter (no 16-bit speedup on Act), so
  everything else is arranged to hide under it; Pool gets only SBUF-to-SBUF
  work (hardware forbids GPSIMD PSUM access) and PSUM evacuations go to
  DVE/Act with partition bases kept at 0/64.
"""

import numpy as np
import ml_dtypes

import concourse.bass as bass
import concourse.tile as tile
from concourse import bacc, mybir
from concourse.bass_utils import run_bass_kernel_spmd

BF16 = mybir.dt.bfloat16
F32 = mybir.dt.float32
AF = mybir.ActivationFunctionType

NCORES = 8
B = 32
BPC = B // NCORES          # batches per core
N = 577                    # sequence length
C = 768
H = 12
HD = 64
R = BPC * N                # rows per core (2308)
CT = C // 128              # 6 contraction tiles
MT = 12                    # qk output row-tiles (1536/128)
JTS = [128, 128, 128, 128, 65]   # j tiles of N
ACOLS = 1024               # T-A: j-tiles 0,1 x i 0:512
BCOLS = 1861               # T-B: j-tiles 2,3,4 x i 0:512 (1536) + c1 5*65
ECOLS = ACOLS + BCOLS      # 2885 packed score/E columns
RCHUNKS = [(0, 512), (512, 512), (1024, 512), (1536, 512), (2048, 260)]
MULB = 768                 # DVE does E-B mul cols [0:MULB], Pool the rest

_PROGRAM = None


def build_program():
    nc = bacc.Bacc(trn_type="TRN2", name="beit_attn")

    xt_d = nc.dram_tensor("xt", [128, CT * R], BF16, kind="ExternalInput")
    # q/k weights m-major ([128, 12 m-tiles x (6 kk x 128)]), v weights kk-major
    wqk_d = nc.dram_tensor("wqk", [128, MT * C], BF16, kind="ExternalInput")
    wv_d = nc.dram_tensor("wv", [128, CT * C], BF16, kind="ExternalInput")
    wp_d = nc.dram_tensor("wp", [128, CT * C], BF16, kind="ExternalInput")
    eb_d = nc.dram_tensor("expb", [H, 128, ECOLS], BF16, kind="ExternalInput")
    id_d = nc.dram_tensor("ident", [128, 128], BF16, kind="ExternalInput")
    pb_d = nc.dram_tensor("pb", [128, CT], F32, kind="ExternalInput")
    out_d = nc.dram_tensor("ftout", [128, CT * R], F32, kind="ExternalOutput")

    with tile.TileContext(nc) as tc:
        with (
            tc.tile_pool(name="static", bufs=1) as sp,
            tc.tile_pool(name="qk", bufs=1) as qk_pool,
            tc.tile_pool(name="v1", bufs=1) as v1_pool,
        ):
            ident = sp.tile([128, 128], BF16, tag="ident")
            pb = sp.tile([128, CT], F32, tag="pb")
            wp_sb = sp.tile([128, CT * C], BF16, tag="wp")
            ot_sb = sp.tile([128, CT * R], BF16, tag="ot")
            # First head's exp-bias staged from the long-lived pool so phase 2
            # doesn't wait on the phase-1 pool-release barrier. DMAs for these
            # are emitted below, sequenced so the phase-1 inputs land first
            # (the DMA fabric is a single shared-bandwidth resource).
            eb0_sb = sp.tile([128, ECOLS], BF16, tag="eb0")

            qk_t = [qk_pool.tile([128, R], BF16, tag=f"qk{m}", name=f"qk{m}") for m in range(MT)]
            v1_t = [
                [v1_pool.tile([128, 780], BF16, tag=f"v1_{b}_{t}", name=f"v1_{b}_{t}") for t in range(5)]
                for b in range(BPC)
            ]

            # ---------------- Phase 1: QKV projections -------------------
            # p1b holds what the interleaved qkT tail (emitted inside phase 2)
            # still needs: x chunks + the last four q/k weight tiles.
            LATE_M = (1, 7, 2, 8, 3, 9, 4, 10, 5, 11)
            p1b_cm = tc.tile_pool(name="ph1b", bufs=1)
            p1b = p1b_cm.__enter__()
            xt_sb = [
                [
                    p1b.tile([128, 1089], BF16, tag=f"xta{kk}", name=f"xta{kk}"),
                    p1b.tile([128, R - 1024], BF16, tag=f"xtb{kk}", name=f"xtb{kk}"),
                ]
                for kk in range(CT)
            ]

            # the halves overlap on cols 1024:1089 so no engine read straddles
            def xt_ap(kk, c0, cn):
                if c0 + cn <= 1089:
                    return xt_sb[kk][0][:, c0 : c0 + cn]
                return xt_sb[kk][1][:, c0 - 1024 : c0 - 1024 + cn]
            wqk_sb = {}
            for m in LATE_M:
                wqk_sb[m] = p1b.tile([128, C], BF16, tag=f"wqk{m}", name=f"wqk{m}")
            with (
                tc.tile_pool(name="ph1", bufs=1) as p1,
                tc.tile_pool(name="psum1", bufs=4, space="PSUM") as psum1,
            ):
                for m in range(MT):
                    if m not in wqk_sb:
                        wqk_sb[m] = p1.tile([128, C], BF16, tag=f"wqk{m}", name=f"wqk{m}")
                wv_sb = [
                    p1.tile([128, C], BF16, tag=f"wv{kk}", name=f"wv{kk}")
                    for kk in range(CT)
                ]

                def dma_wqk(m):
                    nc.sync.dma_start(wqk_sb[m][:], wqk_d[:, C * m : C * (m + 1)])

                # DMA order = arrival order; transfers issued from SP and
                # Act run on separate queues concurrently, so the critical
                # inputs are split across both engines.
                dma_wqk(0)
                nc.scalar.dma_start(wqk_sb[6][:], wqk_d[:, C * 6 : C * 7])
                for kk in range(CT):
                    nc.sync.dma_start(xt_sb[kk][0][:], xt_d[:, R * kk : R * kk + 1089])
                    nc.scalar.dma_start(
                        xt_sb[kk][1][:], xt_d[:, R * kk + 1024 : R * (kk + 1)]
                    )
                for kk in range(CT):
                    eng = nc.sync if kk % 2 == 0 else nc.scalar
                    eng.dma_start(wv_sb[kk][:], wv_d[:, C * kk : C * (kk + 1)])
                nc.scalar.dma_start(eb0_sb[:], eb_d[0])
                nc.sync.dma_start(ident[:], id_d[:])
                nc.sync.dma_start(pb[:], pb_d[:])
                for i, m in enumerate(LATE_M):
                    eng = nc.sync if i % 2 == 0 else nc.scalar
                    eng.dma_start(wqk_sb[m][:], wqk_d[:, C * m : C * (m + 1)])
                nc.sync.dma_start(wp_sb[:], wp_d[:])

                evac_flip = [0]

                def emit_qkT(m):
                    for c0, cn in RCHUNKS:
                        ps = psum1.tile([128, 512], F32, tag="ps1")
                        for kk in range(CT):
                            nc.tensor.matmul(
                                ps[:, :cn],
                                wqk_sb[m][:, 128 * kk : 128 * (kk + 1)],
                                xt_ap(kk, c0, cn),
                                start=(kk == 0),
                                stop=(kk == CT - 1),
                            )
                        evac_flip[0] ^= 1
                        if evac_flip[0]:
                            nc.vector.tensor_copy(qk_t[m][:, c0 : c0 + cn], ps[:, :cn])
                        else:
                            nc.scalar.copy(qk_t[m][:, c0 : c0 + cn], ps[:, :cn])

                def emit_v(b):
                    for t in range(5):
                        kj = JTS[t]
                        for vc0, vcn, h0 in ((0, 512, 0), (512, 256, 8)):
                            ps = psum1.tile([128, 512], F32, tag="ps1")
                            for kk in range(CT):
                                nc.tensor.matmul(
                                    ps[:kj, :vcn],
                                    xt_ap(kk, N * b + 128 * t, kj),
                                    wv_sb[kk][:, vc0 : vc0 + vcn],
                                    start=(kk == 0),
                                    stop=(kk == CT - 1),
                                )
                            dest = v1_t[b][t][:kj, 65 * h0 : 65 * (h0 + vcn // 64)]
                            dest = dest.rearrange("p (h d) -> p h d", d=65)[:, :, 0:64]
                            src = ps[:kj, :vcn].rearrange("p (h d) -> p h d", d=64)
                            nc.vector.tensor_copy(dest, src)
                        ones_ap = v1_t[b][t][:, :].rearrange("p (h d) -> p h d", d=65)[
                            :, :, 64:65
                        ]
                        nc.gpsimd.memset(ones_ap, 1.0)

                # q/k for head pair 0, then v; the late m-tiles (LATE_M)
                # run inside phase 2's PE slack.
                for m in (0, 6):
                    emit_qkT(m)
                for b in range(BPC):
                    emit_v(b)

            # ---------------- Phase 2: attention -------------------------
            with (
                tc.tile_pool(name="ph2", bufs=2) as p2,
                tc.tile_pool(name="ph2sb", bufs=1) as score_pool_sb,
                tc.tile_pool(name="score", bufs=1, space="PSUM") as score_pool,
                tc.tile_pool(name="pvp", bufs=1, space="PSUM") as pv_pool,
                tc.tile_pool(name="ps1b", bufs=1, space="PSUM") as ps1b_pool,
            ):
                ps1b = ps1b_pool.tile([128, 512], F32, tag="ps1b")
                # half-width chunks keep each insertion small; deadline-aware
                # greedy plan: all chunks of head-pair hp must be evacuated
                # before iteration 8*hp reads them.
                RCH2 = [(c, 256) for c in range(0, 2048, 256)] + [(2048, 130), (2178, 130)]
                late_chunks = [
                    (8 * (m % 6) - (2 if m % 6 <= 2 else 3), m, c0, cn, i % 2)
                    for m in LATE_M
                    for i, (c0, cn) in enumerate(RCH2)
                ]
                late_chunks.sort(key=lambda t: t[0])
                plan = [0] * 48
                for dl, _m, _c, _n, _p in late_chunks:
                    slot = min(dl, 47)
                    while slot >= 0 and plan[slot] >= 3:
                        slot -= 1
                    assert slot >= 0, "interleave schedule infeasible"
                    plan[slot] += 1
                # smooth: pull work earlier (never later, so deadlines hold)
                for idx in range(48):
                    while plan[idx] > 2:
                        cands = [j for j in range(idx) if plan[j] < 2]
                        if not cands:
                            break
                        plan[cands[0]] += 1
                        plan[idx] -= 1
                late_flip = [0]

                pend = [None]

                def emit_late_chunk():
                    if not late_chunks:
                        return
                    _dl, m, c0, cn, par = late_chunks.pop(0)
                    pcol = 0 if par == 0 else pend[0][2]
                    for kk in range(CT):
                        nc.tensor.matmul(
                            ps1b[:, pcol : pcol + cn],
                            wqk_sb[m][:, 128 * kk : 128 * (kk + 1)],
                            xt_ap(kk, c0, cn),
                            start=(kk == 0),
                            stop=(kk == CT - 1),
                        )
                    if par == 0:
                        pend[0] = (m, c0, cn)
                    else:
                        pm, pc0, pcn = pend[0]
                        nc.vector.tensor_copy(
                            qk_t[m][:, pc0 : pc0 + pcn + cn],
                            ps1b[:, 0 : pcn + cn],
                        )
                TA = score_pool.tile([128, ACOLS], F32, tag="TA")
                TB = score_pool.tile([128, 2048], F32, tag="TB")
                pv = pv_pool.tile([128, 512], F32, tag="pv")
                # One-time init: j-tile-4 regions rows 65:128 are never written
                # by the M=65 matmuls; zero them so exp stays finite (the expb
                # j-padding rows are 0 so E becomes 0 there). pv rows 65:128 of
                # the s=4 group never get written; 1.0 keeps the strided
                # reciprocal finite. stg pad columns feed the XBAR transpose,
                # so they must be finite too; both rotating buffers get one
                # memset.
                nc.vector.memset(TB[64:128, 1024:1536], 0.0)
                nc.vector.memset(TB[64:128, 1796:1861], 0.0)
                nc.vector.memset(pv[64:128, 260:325], 1.0)
                stg_t = [score_pool_sb.tile([128, 640], BF16, tag=f"stg{i}", name=f"stg{i}") for i in range(2)]
                xscr_t = [score_pool_sb.tile([128, 640], BF16, tag=f"xscr{i}", name=f"xscr{i}") for i in range(2)]
                for i in range(2):
                    nc.gpsimd.memset(stg_t[i][:, :], 0.0)

                prev = [None]

                def emit_tail(st):
                    h, b, par, E0A, E0B, eb_t = st
                    qp = 64 * (h % 2)
                    # E = E0 * exp(bias): A-half on DVE, B-half split DVE/Pool
                    EA = score_pool_sb.tile([128, ACOLS], BF16, tag="EA")
                    EB = score_pool_sb.tile([128, BCOLS], BF16, tag="EB")
                    nc.vector.tensor_mul(EA[:, :], E0A[:, :], eb_t[:, 0:ACOLS])
                    # B-half split jt-aligned so PV's j-tile-2 matmul unblocks
                    # as soon as its own slice is multiplied.
                    nc.gpsimd.tensor_mul(
                        EB[:, 0:512], E0B[:, 0:512], eb_t[:, ACOLS : ACOLS + 512]
                    )
                    nc.gpsimd.tensor_mul(
                        EB[:, 512:1024], E0B[:, 512:1024], eb_t[:, ACOLS + 512 : ACOLS + 1024]
                    )
                    nc.gpsimd.tensor_mul(
                        EB[:, 1024:BCOLS], E0B[:, 1024:BCOLS], eb_t[:, ACOLS + 1024 : ECOLS]
                    )
                    # PV: E stationary, [v|1] moving -> O[i, 65] per i-tile
                    for s in range(5):
                        ki = JTS[s]
                        for jt in range(5):
                            kj = JTS[jt]
                            if s < 4:
                                lhs = (
                                    EA[:kj, 512 * jt + 128 * s : 512 * jt + 128 * s + ki]
                                    if jt < 2
                                    else EB[
                                        :kj,
                                        512 * (jt - 2) + 128 * s : 512 * (jt - 2) + 128 * s + ki,
                                    ]
                                )
                            else:
                                lhs = EB[:kj, 1536 + 65 * jt : 1536 + 65 * jt + ki]
                            nc.tensor.matmul(
                                pv[:ki, 65 * s : 65 * s + 65],
                                lhs,
                                v1_t[b][jt][:kj, 65 * h : 65 * h + 65],
                                start=(jt == 0),
                                stop=(jt == 4),
                            )
                    # per-partition denominators -> reciprocal -> normalize
                    rcp = p2.tile([128, 8], F32, tag="rcp")
                    den_ap = pv[:, 0:325].rearrange("p (s c) -> p s c", c=65)[:, :, 64:65]
                    nc.vector.reciprocal_approx_fast(rcp[:, 0:5], den_ap)
                    # stg groups are 128 wide (64 valid + 64 memset pad) so the
                    # XBAR DMA transpose gets its 128-col alignment. Persistent
                    # parity-rotated tiles keep the pad memset valid.
                    stg = stg_t[par]
                    for s in range(5):
                        ki = JTS[s]
                        nc.vector.tensor_scalar_mul(
                            stg[:ki, 128 * s : 128 * s + 64],
                            pv[:ki, 65 * s : 65 * s + 64],
                            rcp[:ki, s : s + 1],
                        )
                    # O[i, d] -> O^T[d, i] via DMA crossbar transpose into an
                    # SBUF scratch (garbage in rows 64:128 from the pad cols),
                    # then 2 batched Pool copies of the valid rows into ot_sb.
                    obase = R * (h // 2) + N * b
                    xscr = xscr_t[par]
                    for s in range(5):
                        nc.sync.dma_start_transpose(
                            xscr[:, 128 * s : 128 * (s + 1)],
                            stg[:, 128 * s : 128 * (s + 1)],
                        )
                    xv = xscr[:, :].rearrange("p (s c) -> p s c", c=128)
                    nc.gpsimd.tensor_copy(
                        ot_sb[qp : qp + 64, obase : obase + 512],
                        xv[0:64, 0:4, :],
                    )
                    nc.gpsimd.tensor_copy(
                        ot_sb[qp : qp + 64, obase + 512 : obase + 577],
                        xscr[0:64, 512:577],
                    )

                for h in range(H):
                    if h == 0:
                        eb_t = eb0_sb
                    else:
                        eb_t = p2.tile([128, ECOLS], BF16, tag="eb")
                        nc.sync.dma_start(eb_t[:], eb_d[h])
                    qp = 64 * (h % 2)
                    qm, km = h // 2, 6 + h // 2
                    for b in range(BPC):
                        q_c0 = qk_t[qm][qp : qp + 64, N * b : N * b + 512]
                        q_c1 = qk_t[qm][qp : qp + 64, N * b + 512 : N * b + 577]
                        kTs = [
                            qk_t[km][qp : qp + 64, N * b + 128 * jt : N * b + 128 * jt + JTS[jt]]
                            for jt in range(5)
                        ]
                        # T-A: j-tiles 0,1 c0
                        for jt in range(2):
                            nc.tensor.matmul(
                                TA[: JTS[jt], 512 * jt : 512 * jt + 512], kTs[jt], q_c0,
                                start=True, stop=True,
                            )
                        idx = 4 * h + b
                        nslots = plan[idx]
                        if nslots >= 1:
                            emit_late_chunk()
                        # T-B: j-tiles 2,3,4 c0 + all c1
                        for jt in range(2, 5):
                            nc.tensor.matmul(
                                TB[: JTS[jt], 512 * (jt - 2) : 512 * (jt - 2) + 512],
                                kTs[jt], q_c0,
                                start=True, stop=True,
                            )
                        for jt in range(5):
                            nc.tensor.matmul(
                                TB[: JTS[jt], 1536 + 65 * jt : 1536 + 65 * jt + 65],
                                kTs[jt], q_c1,
                                start=True, stop=True,
                            )
                        for _ in range(nslots - 1):
                            emit_late_chunk()
                        E0A = p2.tile([128, ACOLS], BF16, tag="E0A")
                        E0B = p2.tile([128, BCOLS], BF16, tag="E0B")
                        nc.scalar.activation(E0A[:, :], TA[:, :], AF.Exp)
                        nc.scalar.activation(E0B[:, :], TB[:, 0:BCOLS], AF.Exp)
                        if prev[0] is not None:
                            emit_tail(prev[0])
                        prev[0] = (h, b, (4 * h + b) % 2, E0A, E0B, eb_t)
                emit_tail(prev[0])

            p1b_cm.__exit__(None, None, None)

            # ---------------- Phase 3: output projection -----------------
            with (
                tc.tile_pool(name="ph3", bufs=6) as p3,
                tc.tile_pool(name="psum3", bufs=8, space="PSUM") as psum3,
            ):
                for m in range(CT):
                    for c0, cn in RCHUNKS:
                        ps = psum3.tile([128, 512], F32, tag="ps3")
                        for kk in range(CT):
                            nc.tensor.matmul(
                                ps[:, :cn],
                                wp_sb[:, C * kk + 128 * m : C * kk + 128 * (m + 1)],
                                ot_sb[:, R * kk + c0 : R * kk + c0 + cn],
                                start=(kk == 0),
                                stop=(kk == CT - 1),
                            )
                        ft = p3.tile([128, 512], F32, tag="ft")
                        nc.scalar.add(ft[:, :cn], ps[:, :cn], pb[:, m : m + 1])
                        deng = nc.sync if (m + c0 // 512) % 2 == 0 else nc.scalar
                        deng.dma_start(
                            out_d[:, R * m + c0 : R * m + c0 + cn], ft[:, :cn]
                        )

    nc.compile()
    return nc


def get_program():
    global _PROGRAM
    if _PROGRAM is None:
        _PROGRAM = build_program()
    return _PROGRAM


def _pack_ctiles(a):
    """[768, X] -> SBUF image [128, 6*X] (c-tile kk at cols kk*X..(kk+1)*X)."""
    rows, cols = a.shape
    assert rows == 768
    return np.ascontiguousarray(
        a.reshape(CT, 128, cols).transpose(1, 0, 2).reshape(128, CT * cols)
    )


def make_host_inputs(x, qkv_w, table, rel_index, proj_w, proj_b):
    bf = ml_dtypes.bfloat16
    x = np.asarray(x, np.float32)
    qkv_w = np.asarray(qkv_w, np.float32)
    table = np.asarray(table, np.float32)
    rel_index = np.asarray(rel_index)
    proj_w = np.asarray(proj_w, np.float32)
    proj_b = np.asarray(proj_b, np.float32)

    qkv_ws = qkv_w.copy()
    qkv_ws[:768] *= 0.125                                    # fold q scale (exact in bf16)
    wqkv = _pack_ctiles(qkv_ws.T)                            # [128, 6*2304]
    # q/k weights m-major: tile m holds cols [6 kk x 128] for output rows
    # 128m..128(m+1); v weights stay kk-major.
    wqkv3 = wqkv.reshape(128, CT, 18, 128)
    wqk = np.ascontiguousarray(
        wqkv3[:, :, :MT].transpose(0, 2, 1, 3).reshape(128, MT * C)
    ).astype(bf)
    wv = np.ascontiguousarray(wqkv3[:, :, MT:].reshape(128, CT * C)).astype(bf)
    wp = _pack_ctiles(proj_w.T).astype(bf)                   # [128, 6*768]
    pb = np.ascontiguousarray(proj_b.reshape(CT, 128).T)     # [128, 6]
    ident = np.eye(128, dtype=bf)

    # exp(bias), transposed orientation: ebT[h, j, i] = exp(table[rel_index[i, j], h])
    g = table[rel_index.reshape(-1)].reshape(N, N, H)        # [i, j, h]
    bt = np.exp(g.transpose(2, 1, 0))                        # [h, j, i]
    btp = np.zeros((H, 640, N), np.float32)                  # j-padding rows stay 0
    btp[:, :N] = bt
    btp = btp.reshape(H, 5, 128, N)
    c0 = btp[:, :, :, 0:512].transpose(0, 2, 1, 3).reshape(H, 128, 2560)
    c1 = btp[:, :, :, 512:577].transpose(0, 2, 1, 3).reshape(H, 128, 325)
    # device column order: [jt0 c0, jt1 c0 | jt2-4 c0, c1 x5] = A(1024) + B(1861)
    expb = np.ascontiguousarray(
        np.concatenate([c0[:, :, 0:1024], c0[:, :, 1024:2560], c1], axis=2)
    ).astype(bf)

    in_maps = []
    for c in range(NCORES):
        xT = x[BPC * c : BPC * (c + 1)].reshape(R, C).T      # [768, 2308]
        in_maps.append(
            {
                "xt": _pack_ctiles(xT).astype(bf),
                "wqk": wqk,
                "wv": wv,
                "wp": wp,
                "expb": expb,
                "ident": ident,
                "pb": pb,
            }
        )
    return in_maps


def unpack_output(ft):
    """[128, 6*2308] f32 -> [BPC, 577, 768]."""
    f = ft.reshape(128, CT, R).transpose(1, 0, 2).reshape(C, R)  # [768, 2308]
    return np.ascontiguousarray(f.T).reshape(BPC, N, C)


def kernel(x, qkv_w, table, rel_index, proj_w, proj_b):
    nc = get_program()
    in_maps = make_host_inputs(x, qkv_w, table, rel_index, proj_w, proj_b)
    res = run_bass_kernel_spmd(nc, in_maps, core_ids=list(range(NCORES)))
    out = np.empty((B, N, C), np.float32)
    for c in range(NCORES):
        out[BPC * c : BPC * (c + 1)] = unpack_output(res.results[c]["ftout"])
    return out
